# revision 5
# baseline (speedup 1.0000x reference)
"""Fuzzy-attention 2-layer GCN on 8 trn2 NeuronCores (Bass/Tile).

Strategy (dst-sharding, node-major CSR):
  - Nodes are split across the 8 cores by destination; each core owns 12544
    nodes (N padded 100000 -> 100352). Within a core, nodes are sorted by
    in-degree so 128-node blocks have near-uniform slot counts.
  - Per block, a padded CSR grid [128 nodes x S_b slots] holds each node's
    incoming edges (src id, per-edge weight/mu/sigma terms). Pad slots carry
    weight 0.
  - Layer 1 per block: indirect-DMA gather feat[src] -> [128, S_b, 32],
    stream feat[dst] (contiguous, host-permuted), compute the fuzzy
    attention per slot (dist via ACT ln/exp, gaussian via ACT exp), combine
    into per-edge coefficients att*nsrc*ndst, multiply + slot-reduce on DVE,
    append bias column, transpose on PE, matmul with W1 (bias folded),
    ReLU -> x1 shard.
  - Host concatenates x1 shards (pure resharding) and launches layer 2:
    same grids gather x1[src], multiply by stored coefficients, reduce,
    W2+bias, ReLU, then masked on-device partial reductions (sum/max/min
    excluding nodes 0,1 and padding). Host combines 8 partial triples and
    rows 0,1 -> [5, 64] output.

GNN_MODE env var: "dev" (default) = device indirect gathers;
"host" = host pre-gathers feature rows, device streams them (fallback).
"""
import os
import sys
import re
import types
import contextlib
import ctypes

sys.path.insert(0, "/opt/trn_rl_repo")

import numpy as np

# ---------------------------------------------------------------- tile fixes
import concourse.tile_sem_assignment as _tsa
_tsa.NUM_SWDGE_GLOBAL_SEMS = 2

import concourse.bass as bass
import concourse.mybir as mybir
import concourse.tile as tile
from concourse.vector_clock import ScopedClock, VectorClock
from concourse.bass_utils import run_bass_kernel_spmd


def _patched_drain_and_barrier(self, tick_clock, wait_clock):
    # This walrus build allows at most ONE sem wait on the tail drain.
    # Absorb every proc's final tick with single-wait sync NOPs first.
    nc = self.nc
    gc = tick_clock.global_clock
    vals = [int(x) for x in re.findall(r"\d+", str(gc))]
    for proc, v in enumerate(vals):
        if v > 0:
            pc = VectorClock()
            pc.require_at_least(proc, v)
            nop = nc.sync.nop(nofuse=True, hint="tailfix")
            wait_clock.add_sem_waits(nop.ins, ScopedClock({None: pc}))
    nc.sync.drain()
    nc.all_engine_barrier()
    assert self.sems is not None
    popped = nc._tile_sem_poison_stack.pop()
    assert popped is self._sem_poison
    nc.clear_and_free_semaphores(list(self.sems.allocated().values()))
    nc.all_engine_barrier()


tile.TileContext._drain_and_barrier = _patched_drain_and_barrier

# This walrus build also rejects ANY instruction carrying more than one sem
# wait. Post-process the serialized BIR: hoist all-but-one waits of each
# instruction onto injected same-engine NoOps (engines execute in order, so
# waiting earlier on the same engine is semantically identical).
try:
    import orjson as _oj

    def _loads(b):
        return _oj.loads(b)

    def _dumps(m):
        return _oj.dumps(m)
except ImportError:
    import json as _j

    def _loads(b):
        return _j.loads(b)

    def _dumps(m):
        return _j.dumps(m).encode()


def _hoist_multiwaits(mod):
    changed = False
    k = 0
    for fn in mod.get("functions", []):
        for blk in fn.get("blocks", []):
            insts = blk.get("instructions", [])
            out = []
            for inst in insts:
                si = inst.get("sync_info")
                waits = (si or {}).get("on_wait") or []
                if len(waits) > 1:
                    changed = True
                    for w in waits[:-1]:
                        out.append({
                            "debug": inst.get("debug", 0),
                            "engine": inst["engine"],
                            "ins": [],
                            "name": f"I-mwfix{k}",
                            "opcode": "NoOp",
                            "outs": [],
                            "sync_info": {"on_update": [], "on_wait": [w]},
                        })
                        k += 1
                    si["on_wait"] = [waits[-1]]
                out.append(inst)
            blk["instructions"] = out
    return changed


_orig_to_json_bytes = bass.Bass.to_json_bytes


def _patched_to_json_bytes(self):
    raw = _orig_to_json_bytes(self)
    mod = _loads(raw)
    if _hoist_multiwaits(mod):
        return _dumps(mod)
    return raw


bass.Bass.to_json_bytes = _patched_to_json_bytes

# ------------------------------------------------------- ntff profiling hook
# (needed only when trace=True; harmless otherwise)
_SO_PATH = "/opt/axon/libaxon_pjrt.so"


def _ntff_profile_via_ctypes(so_path):
    try:
        lib = ctypes.CDLL(so_path)
    except OSError:
        return None
    if not hasattr(lib, "axon_start_nrt_profile"):
        return None
    lib.axon_start_nrt_profile.argtypes = [ctypes.POINTER(ctypes.c_int64), ctypes.c_size_t]
    lib.axon_start_nrt_profile.restype = ctypes.c_int64
    lib.axon_stop_nrt_profile.argtypes = [ctypes.c_char_p]
    lib.axon_stop_nrt_profile.restype = ctypes.c_int64

    @contextlib.contextmanager
    def _hook(output_dir, device_ids):
        import jax
        jax.devices()
        if device_ids:
            ids = (ctypes.c_int64 * len(device_ids))(*device_ids)
            rc = lib.axon_start_nrt_profile(ids, len(device_ids))
        else:
            rc = lib.axon_start_nrt_profile(None, 0)
        if rc != 0:
            raise RuntimeError(f"axon_start_nrt_profile rc={rc}")
        try:
            yield
        finally:
            n = lib.axon_stop_nrt_profile(str(output_dir).encode())
            if n <= 0:
                print(f"ntff profile: wrote {n} files to {output_dir}", file=sys.stderr)

    return _hook


if "antenv.axon_hooks" not in sys.modules:
    _mod = types.ModuleType("antenv.axon_hooks")
    _mod._hook = _ntff_profile_via_ctypes(_SO_PATH)
    _mod.set_axon_ntff_profile_hook = lambda h: setattr(_mod, "_hook", h)
    _mod.get_axon_ntff_profile_hook = lambda: _mod._hook
    sys.modules["antenv.axon_hooks"] = _mod

# ---------------------------------------------------------------- constants
P = 128
N_CORES = 8
BIG = 1.0e30
F32 = mybir.dt.float32
I32 = mybir.dt.int32
AF = mybir.ActivationFunctionType
OP = mybir.AluOpType
AX = mybir.AxisListType

TRACE = [None, None]  # exec_time_ns for launch1/launch2 of the last call


# =================================================================== host prep
def _prep(feat, src, dst, etypes, W1, b1, W2, b2, mu, sigma):
    N, IN = feat.shape
    E = src.shape[0]
    H = W1.shape[1]
    OUT = W2.shape[1]
    npc_raw = -(-N // (N_CORES * P))          # blocks per core
    NB = npc_raw
    NPC = NB * P                              # nodes per core
    NPAD = NPC * N_CORES

    src = np.asarray(src, np.int64)
    dst = np.asarray(dst, np.int64)
    etypes = np.asarray(etypes, np.int64)

    deg_out = np.bincount(src, minlength=N).astype(np.int64)
    deg_in = np.bincount(dst, minlength=N).astype(np.int64)
    nsrc = 1.0 / np.sqrt(np.maximum(deg_out, 1).astype(np.float64))
    ndst = 1.0 / np.sqrt(np.maximum(deg_in, 1).astype(np.float64))

    # --- node -> core assignment: deg-desc snake, nodes 0,1 pinned to core 0
    deg_full = np.concatenate([deg_in, np.zeros(NPAD - N, np.int64)])
    rest = np.setdiff1d(np.arange(NPAD), [0, 1], assume_unique=False)
    rest = rest[np.argsort(-deg_full[rest], kind="stable")]
    lists = [[0, 1]] + [[] for _ in range(N_CORES - 1)]
    fwd = True
    i = 0
    while i < len(rest):
        order = range(N_CORES) if fwd else range(N_CORES - 1, -1, -1)
        for c in order:
            if len(lists[c]) < NPC and i < len(rest):
                lists[c].append(int(rest[i]))
                i += 1
        fwd = not fwd
    nodes_by_core = [np.array(l, np.int64) for l in lists]
    assert all(len(l) == NPC for l in nodes_by_core)

    perm = np.concatenate(nodes_by_core)                  # permuted id -> orig id
    prank = np.empty(NPAD, np.int64)
    prank[perm] = np.arange(NPAD)

    # --- per-(core, block-position) slot counts, unified across cores
    S_b = np.zeros(NB, np.int64)
    for c in range(N_CORES):
        dg = deg_full[nodes_by_core[c]].reshape(NB, P)
        S_b = np.maximum(S_b, dg.max(axis=1))
    S_b = np.maximum(S_b, 1).astype(np.int64)
    offs = np.zeros(NB + 1, np.int64)
    offs[1:] = np.cumsum(S_b)
    S_tot = int(offs[-1])

    # --- CSR over dst
    esort = np.argsort(dst, kind="stable")
    row_start = np.zeros(N + 1, np.int64)
    row_start[1:] = np.cumsum(deg_in)

    # per-edge terms
    mu_e = mu[etypes].astype(np.float32)
    i2neg_e = (-1.0 / (2.0 * (sigma.astype(np.float64) ** 2)))[etypes].astype(np.float32)
    w_e = (nsrc[src] * ndst[dst]).astype(np.float32)

    grids = []
    for c in range(N_CORES):
        nodes = nodes_by_core[c]
        src_g = np.zeros((P, S_tot), np.int32)
        w_g = np.zeros((P, S_tot), np.float32)
        mu_g = np.zeros((P, S_tot), np.float32)
        i2_g = np.zeros((P, S_tot), np.float32)
        # vectorized-ish per block
        for b in range(NB):
            blk = nodes[b * P:(b + 1) * P]
            degs = np.where(blk < N, deg_full[blk], 0)
            tot = int(degs.sum())
            if tot == 0:
                continue
            pp = np.repeat(np.arange(P), degs)
            ss = np.arange(tot) - np.repeat(np.cumsum(degs) - degs, degs)
            ee = np.concatenate([
                esort[row_start[v]:row_start[v] + deg_full[v]]
                for v in blk if v < N and deg_full[v] > 0
            ]) if tot else np.zeros(0, np.int64)
            cols = offs[b] + ss
            src_g[pp, cols] = prank[src[ee]].astype(np.int32)
            w_g[pp, cols] = w_e[ee]
            mu_g[pp, cols] = mu_e[ee]
            i2_g[pp, cols] = i2neg_e[ee]
        grids.append(dict(src=src_g, w=w_g, mu=mu_g, i2=i2_g))

    feat_pad = np.zeros((NPAD, IN), np.float32)
    feat_pad[:N] = feat
    feat_perm = feat_pad[perm]                               # [NPAD, IN]

    # masks for final reductions: valid real node, not 0/1
    mask = np.zeros((N_CORES, P, NB), np.float32)
    for c in range(N_CORES):
        v = nodes_by_core[c].reshape(NB, P).T                # [P, NB]
        mask[c] = ((v < N) & (v >= 2)).astype(np.float32)

    W1a = np.vstack([W1.astype(np.float32), b1.astype(np.float32)[None, :]])  # [33,64]
    W2a = np.vstack([W2.astype(np.float32), b2.astype(np.float32)[None, :]])  # [65,64]

    return dict(
        N=N, E=E, IN=IN, H=H, OUT=OUT, NB=NB, NPC=NPC, NPAD=NPAD,
        S_b=S_b, offs=offs, S_tot=S_tot, grids=grids, perm=perm,
        prank=prank, feat_perm=feat_perm, mask=mask, W1a=W1a, W2a=W2a,
        nodes_by_core=nodes_by_core,
    )


# ============================================================ bass builders
def _build_l1(pp, mode):
    NB, S_tot, IN, H = pp["NB"], pp["S_tot"], pp["IN"], pp["H"]
    NPC, NPAD = pp["NPC"], pp["NPAD"]
    S_b, offs = pp["S_b"], pp["offs"]
    KA = IN + 1

    nc = bass.Bass()
    g_src = nc.declare_dram_parameter("g_src", [P, S_tot], I32, isOutput=False)
    g_w = nc.declare_dram_parameter("g_w", [P, S_tot], F32, isOutput=False)
    g_mu = nc.declare_dram_parameter("g_mu", [P, S_tot], F32, isOutput=False)
    g_i2 = nc.declare_dram_parameter("g_i2", [P, S_tot], F32, isOutput=False)
    featp = nc.declare_dram_parameter("featp", [NPAD, IN], F32, isOutput=False)
    featd = nc.declare_dram_parameter("featd", [NPC, IN], F32, isOutput=False)
    w1a = nc.declare_dram_parameter("w1a", [KA, H], F32, isOutput=False)
    ones = nc.declare_dram_parameter("ones", [P, 1], F32, isOutput=False)
    ident = nc.declare_dram_parameter("ident", [P, P], F32, isOutput=False)
    if mode == "host":
        fs_pre = nc.declare_dram_parameter("fs_pre", [P, S_tot, IN], F32, isOutput=False)
    x1_out = nc.declare_dram_parameter("x1", [NPC, H], F32, isOutput=True)
    coef_out = nc.declare_dram_parameter("coef", [P, S_tot], F32, isOutput=True)

    with tile.TileContext(nc) as tc:
        with (
            tc.tile_pool(name="persist", bufs=1) as pers,
            tc.tile_pool(name="work", bufs=3) as work,
            tc.tile_pool(name="small", bufs=4) as small,
            tc.tile_pool(name="psum", bufs=2, space="PSUM") as psum,
        ):
            # persistent tiles
            if mode == "dev":
                srcg_t = pers.tile([P, S_tot], I32, tag="srcg")
                nc.sync.dma_start(out=srcg_t[:], in_=g_src[:, :])
            wg_t = pers.tile([P, S_tot], F32, tag="wg")
            nc.sync.dma_start(out=wg_t[:], in_=g_w[:, :])
            mug_t = pers.tile([P, S_tot], F32, tag="mug")
            nc.sync.dma_start(out=mug_t[:], in_=g_mu[:, :])
            i2g_t = pers.tile([P, S_tot], F32, tag="i2g")
            nc.sync.dma_start(out=i2g_t[:], in_=g_i2[:, :])
            w1_t = pers.tile([KA, H], F32, tag="w1")
            nc.sync.dma_start(out=w1_t[:], in_=w1a[:, :])
            ones_t = pers.tile([P, 1], F32, tag="ones")
            nc.sync.dma_start(out=ones_t[:], in_=ones[:, :])
            ident_t = pers.tile([P, P], F32, tag="ident")
            nc.sync.dma_start(out=ident_t[:], in_=ident[:, :])

            for b in range(NB):
                S = int(S_b[b])
                o = int(offs[b])
                fs = work.tile([P, S, IN], F32, tag="fs")
                if mode == "dev":
                    for s in range(S):
                        nc.gpsimd.indirect_dma_start(
                            out=fs[:, s, :], out_offset=None,
                            in_=featp[:, :],
                            in_offset=bass.IndirectOffsetOnAxis(
                                ap=srcg_t[:, o + s:o + s + 1], axis=0),
                        )
                else:
                    nc.sync.dma_start(out=fs[:, :, :], in_=fs_pre[:, o:o + S, :])
                fd = work.tile([P, IN], F32, tag="fd")
                nc.sync.dma_start(out=fd[:], in_=featd[b * P:(b + 1) * P, :])

                diff = work.tile([P, S, IN], F32, tag="diff")
                nc.vector.tensor_tensor(
                    out=diff[:], in0=fs[:],
                    in1=fd[:].unsqueeze(1).to_broadcast([P, S, IN]),
                    op=OP.subtract)
                sq = work.tile([P, S, IN], F32, tag="sq")
                nc.scalar.activation(sq[:], diff[:], AF.Square)
                d2 = small.tile([P, S], F32, tag="d2")
                nc.vector.tensor_reduce(out=d2[:], in_=sq[:], axis=AX.X, op=OP.add)
                lnd = small.tile([P, S], F32, tag="lnd")
                nc.scalar.activation(lnd[:], d2[:], AF.Ln)
                dist = small.tile([P, S], F32, tag="dist")
                nc.scalar.activation(dist[:], lnd[:], AF.Exp, scale=0.5)
                t1 = small.tile([P, S], F32, tag="t1")
                nc.vector.tensor_tensor(out=t1[:], in0=dist[:],
                                        in1=mug_t[:, o:o + S], op=OP.subtract)
                t2 = small.tile([P, S], F32, tag="t2")
                nc.vector.tensor_tensor(out=t2[:], in0=t1[:], in1=t1[:], op=OP.mult)
                arg = small.tile([P, S], F32, tag="arg")
                nc.vector.tensor_tensor(out=arg[:], in0=t2[:],
                                        in1=i2g_t[:, o:o + S], op=OP.mult)
                att = small.tile([P, S], F32, tag="att")
                nc.scalar.activation(att[:], arg[:], AF.Exp)
                coef = small.tile([P, S], F32, tag="coef")
                nc.vector.tensor_tensor(out=coef[:], in0=att[:],
                                        in1=wg_t[:, o:o + S], op=OP.mult)
                nc.sync.dma_start(out=coef_out[:, o:o + S], in_=coef[:])

                msgs = work.tile([P, S, IN], F32, tag="msgs")
                nc.vector.tensor_tensor(
                    out=msgs[:], in0=fs[:],
                    in1=coef[:].unsqueeze(2).to_broadcast([P, S, IN]),
                    op=OP.mult)
                agg = work.tile([P, KA], F32, tag="agg")
                nc.vector.tensor_reduce(
                    out=agg[:, 0:IN],
                    in_=msgs[:].rearrange("p s d -> p d s"),
                    axis=AX.X, op=OP.add)
                nc.vector.tensor_copy(agg[:, IN:IN + 1], ones_t[:])

                aggT_p = psum.tile([KA, P], F32, tag="aggT", space="PSUM")
                nc.tensor.transpose(out=aggT_p[:], in_=agg[:], identity=ident_t[:])
                aggT = work.tile([KA, P], F32, tag="aggTs")
                nc.scalar.copy(aggT[:], aggT_p[:])

                x1_p = psum.tile([P, H], F32, tag="x1p", space="PSUM")
                nc.tensor.matmul(out=x1_p[:], lhsT=aggT[:], rhs=w1_t[:],
                                 start=True, stop=True)
                x1_s = work.tile([P, H], F32, tag="x1s")
                nc.scalar.activation(x1_s[:], x1_p[:], AF.Relu)
                nc.sync.dma_start(out=x1_out[b * P:(b + 1) * P, :], in_=x1_s[:])
    return nc


def _build_l2(pp, mode):
    NB, S_tot, H, OUT = pp["NB"], pp["S_tot"], pp["H"], pp["OUT"]
    NPC, NPAD = pp["NPC"], pp["NPAD"]
    S_b, offs = pp["S_b"], pp["offs"]
    KA = H + 1

    nc = bass.Bass()
    g_src = nc.declare_dram_parameter("g_src", [P, S_tot], I32, isOutput=False)
    g_coef = nc.declare_dram_parameter("g_coef", [P, S_tot], F32, isOutput=False)
    w2a = nc.declare_dram_parameter("w2a", [KA, OUT], F32, isOutput=False)
    ones = nc.declare_dram_parameter("ones", [P, 1], F32, isOutput=False)
    ident = nc.declare_dram_parameter("ident", [P, P], F32, isOutput=False)
    maskm = nc.declare_dram_parameter("maskm", [P, NB], F32, isOutput=False)
    offneg = nc.declare_dram_parameter("offneg", [P, NB], F32, isOutput=False)
    offpos = nc.declare_dram_parameter("offpos", [P, NB], F32, isOutput=False)
    zer64 = nc.declare_dram_parameter("zer64", [P, OUT], F32, isOutput=False)
    nbig64 = nc.declare_dram_parameter("nbig64", [P, OUT], F32, isOutput=False)
    pbig64 = nc.declare_dram_parameter("pbig64", [P, OUT], F32, isOutput=False)
    if mode == "dev":
        x1full = nc.declare_dram_parameter("x1full", [NPAD, H], F32, isOutput=False)
    else:
        x1s_pre = nc.declare_dram_parameter("x1s_pre", [P, S_tot, H], F32, isOutput=False)
    out01 = nc.declare_dram_parameter("out01", [2, OUT], F32, isOutput=True)
    psum_out = nc.declare_dram_parameter("psum_out", [OUT, 1], F32, isOutput=True)
    pmax_out = nc.declare_dram_parameter("pmax_out", [OUT, 1], F32, isOutput=True)
    pmin_out = nc.declare_dram_parameter("pmin_out", [OUT, 1], F32, isOutput=True)

    with tile.TileContext(nc) as tc:
        with (
            tc.tile_pool(name="persist", bufs=1) as pers,
            tc.tile_pool(name="work", bufs=3) as work,
            tc.tile_pool(name="small", bufs=4) as small,
            tc.tile_pool(name="psum", bufs=2, space="PSUM") as psum,
        ):
            if mode == "dev":
                srcg_t = pers.tile([P, S_tot], I32, tag="srcg")
                nc.sync.dma_start(out=srcg_t[:], in_=g_src[:, :])
            coefg_t = pers.tile([P, S_tot], F32, tag="coefg")
            nc.sync.dma_start(out=coefg_t[:], in_=g_coef[:, :])
            w2_t = pers.tile([KA, OUT], F32, tag="w2")
            nc.sync.dma_start(out=w2_t[:], in_=w2a[:, :])
            ones_t = pers.tile([P, 1], F32, tag="ones")
            nc.sync.dma_start(out=ones_t[:], in_=ones[:, :])
            ident_t = pers.tile([P, P], F32, tag="ident")
            nc.sync.dma_start(out=ident_t[:], in_=ident[:, :])
            mask_t = pers.tile([P, NB], F32, tag="maskm")
            nc.sync.dma_start(out=mask_t[:], in_=maskm[:, :])
            offn_t = pers.tile([P, NB], F32, tag="offn")
            nc.sync.dma_start(out=offn_t[:], in_=offneg[:, :])
            offp_t = pers.tile([P, NB], F32, tag="offp")
            nc.sync.dma_start(out=offp_t[:], in_=offpos[:, :])

            sum_acc = pers.tile([P, OUT], F32, tag="sumacc")
            nc.sync.dma_start(out=sum_acc[:], in_=zer64[:, :])
            max_acc = pers.tile([P, OUT], F32, tag="maxacc")
            nc.sync.dma_start(out=max_acc[:], in_=nbig64[:, :])
            min_acc = pers.tile([P, OUT], F32, tag="minacc")
            nc.sync.dma_start(out=min_acc[:], in_=pbig64[:, :])

            for b in range(NB):
                S = int(S_b[b])
                o = int(offs[b])
                g = work.tile([P, S, H], F32, tag="g")
                if mode == "dev":
                    for s in range(S):
                        nc.gpsimd.indirect_dma_start(
                            out=g[:, s, :], out_offset=None,
                            in_=x1full[:, :],
                            in_offset=bass.IndirectOffsetOnAxis(
                                ap=srcg_t[:, o + s:o + s + 1], axis=0),
                        )
                else:
                    nc.sync.dma_start(out=g[:, :, :], in_=x1s_pre[:, o:o + S, :])

                msgs = work.tile([P, S, H], F32, tag="msgs")
                nc.vector.tensor_tensor(
                    out=msgs[:], in0=g[:],
                    in1=coefg_t[:, o:o + S].unsqueeze(2).to_broadcast([P, S, H]),
                    op=OP.mult)
                agg = work.tile([P, KA], F32, tag="agg")
                nc.vector.tensor_reduce(
                    out=agg[:, 0:H],
                    in_=msgs[:].rearrange("p s d -> p d s"),
                    axis=AX.X, op=OP.add)
                nc.vector.tensor_copy(agg[:, H:H + 1], ones_t[:])

                aggT_p = psum.tile([KA, P], F32, tag="aggT", space="PSUM")
                nc.tensor.transpose(out=aggT_p[:], in_=agg[:], identity=ident_t[:])
                aggT = work.tile([KA, P], F32, tag="aggTs")
                nc.scalar.copy(aggT[:], aggT_p[:])

                x2_p = psum.tile([P, OUT], F32, tag="x2p", space="PSUM")
                nc.tensor.matmul(out=x2_p[:], lhsT=aggT[:], rhs=w2_t[:],
                                 start=True, stop=True)
                x2_s = work.tile([P, OUT], F32, tag="x2s")
                nc.scalar.activation(x2_s[:], x2_p[:], AF.Relu)

                if b == 0:
                    nc.sync.dma_start(out=out01[:, :], in_=x2_s[0:2, :])

                x2m = work.tile([P, OUT], F32, tag="x2m")
                nc.vector.tensor_scalar(out=x2m[:], in0=x2_s[:],
                                        scalar1=mask_t[:, b:b + 1], scalar2=None,
                                        op0=OP.mult)
                nc.vector.tensor_tensor(out=sum_acc[:], in0=sum_acc[:],
                                        in1=x2m[:], op=OP.add)
                xmx = work.tile([P, OUT], F32, tag="xmx")
                nc.vector.scalar_tensor_tensor(
                    out=xmx[:], in0=x2_s[:], scalar=mask_t[:, b:b + 1],
                    in1=offn_t[:, b:b + 1].to_broadcast([P, OUT]),
                    op0=OP.mult, op1=OP.add)
                nc.vector.tensor_tensor(out=max_acc[:], in0=max_acc[:],
                                        in1=xmx[:], op=OP.max)
                xmn = work.tile([P, OUT], F32, tag="xmn")
                nc.vector.scalar_tensor_tensor(
                    out=xmn[:], in0=x2_s[:], scalar=mask_t[:, b:b + 1],
                    in1=offp_t[:, b:b + 1].to_broadcast([P, OUT]),
                    op0=OP.mult, op1=OP.add)
                nc.vector.tensor_tensor(out=min_acc[:], in0=min_acc[:],
                                        in1=xmn[:], op=OP.min)

            # ---- epilogue: cross-partition reduce via PE transpose
            for acc, op, dram in ((sum_acc, OP.add, psum_out),
                                  (max_acc, OP.max, pmax_out),
                                  (min_acc, OP.min, pmin_out)):
                tp = psum.tile([OUT, P], F32, tag="finT", space="PSUM")
                nc.tensor.transpose(out=tp[:], in_=acc[:], identity=ident_t[:])
                red = small.tile([OUT, 1], F32, tag="fred")
                nc.vector.tensor_reduce(out=red[:], in_=tp[:], axis=AX.X, op=op)
                nc.sync.dma_start(out=dram[:, :], in_=red[:])
    return nc


# ==================================================================== kernel
def kernel(feat, src, dst, etypes, W1, b1, W2, b2, mu, sigma, _trace=False):
    mode = os.environ.get("GNN_MODE", "dev")
    pp = _prep(feat, src, dst, etypes, W1, b1, W2, b2, mu, sigma)
    NB, NPC, NPAD = pp["NB"], pp["NPC"], pp["NPAD"]
    N, IN, H, OUT = pp["N"], pp["IN"], pp["H"], pp["OUT"]
    S_tot = pp["S_tot"]

    ones = np.ones((P, 1), np.float32)
    core_ids = list(range(N_CORES))

    # ------------- launch 1
    nc1 = _build_l1(pp, mode)
    in_maps = []
    for c in range(N_CORES):
        g = pp["grids"][c]
        m = dict(
            g_src=g["src"], g_w=g["w"], g_mu=g["mu"], g_i2=g["i2"],
            featp=np.ascontiguousarray(pp["feat_perm"]),
            featd=np.ascontiguousarray(
                pp["feat_perm"][c * pp["NPC"]:(c + 1) * pp["NPC"]]),
            w1a=pp["W1a"], ones=ones, ident=np.eye(P, dtype=np.float32),
        )
        if mode == "host":
            fs = pp["feat_perm"][g["src"].astype(np.int64)]       # [P, S_tot, IN]
            m["fs_pre"] = np.ascontiguousarray(fs)
        in_maps.append(m)
    r1 = run_bass_kernel_spmd(nc1, in_maps, core_ids, trace=_trace)
    TRACE[0] = r1.exec_time_ns
    x1_shards = [r1.results[c]["x1"] for c in range(N_CORES)]
    coefs = [r1.results[c]["coef"] for c in range(N_CORES)]
    x1_full = np.concatenate(x1_shards, axis=0)                   # [NPAD, H] perm order

    # ------------- launch 2
    nc2 = _build_l2(pp, mode)
    zer64 = np.zeros((P, OUT), np.float32)
    nbig64 = np.full((P, OUT), -BIG, np.float32)
    pbig64 = np.full((P, OUT), BIG, np.float32)
    in_maps2 = []
    for c in range(N_CORES):
        g = pp["grids"][c]
        mk = pp["mask"][c]
        m = dict(
            g_src=g["src"], g_coef=coefs[c], w2a=pp["W2a"], ones=ones,
            ident=np.eye(P, dtype=np.float32),
            maskm=mk, offneg=((1 - mk) * -BIG).astype(np.float32),
            offpos=((1 - mk) * BIG).astype(np.float32),
            zer64=zer64, nbig64=nbig64, pbig64=pbig64,
        )
        if mode == "dev":
            m["x1full"] = x1_full
        else:
            m["x1s_pre"] = np.ascontiguousarray(x1_full[g["src"].astype(np.int64)])
        in_maps2.append(m)
    r2 = run_bass_kernel_spmd(nc2, in_maps2, core_ids, trace=_trace)
    TRACE[1] = r2.exec_time_ns

    sums = np.stack([r2.results[c]["psum_out"][:, 0] for c in range(N_CORES)])
    maxs = np.stack([r2.results[c]["pmax_out"][:, 0] for c in range(N_CORES)])
    mins = np.stack([r2.results[c]["pmin_out"][:, 0] for c in range(N_CORES)])
    x01 = r2.results[0]["out01"]

    out = np.stack([
        x01[0], x01[1],
        (sums.sum(axis=0) / (N - 2)).astype(np.float32),
        maxs.max(axis=0), mins.min(axis=0),
    ], axis=0).astype(np.float32)
    return out
